# revision 43
# baseline (speedup 1.0000x reference)
"""Trainium2 Bass kernel for nn_AttentionEncoder (B=32, L=577, D=512, H=8, FF=2048).

Strategy: data-parallel over batch across 8 NeuronCores (4 samples/core).
Each core runs the full encoder on its 4 samples in two phases per rep:
  phase ATT (exp+ln ACT table): LN1 -> QKV -> attention -> wo (+res) -> LN2
  phase FFN (gelu ACT table):   FFN (+res) -> transpose back -> store
so the ACT engine loads each activation table once per rep instead of
once per sample.

This environment is issue/latency-bound (per-instruction sequencer
overhead 50-100ns, sem hops ~100-250ns, Pool launch 95ns), so the layout
optimizes instruction count and cross-engine pipelining, not just engine
busy time:
  - phase-ATT emission is software-pipelined pairs(b) -> front(b+1) ->
    tail(b), so sample b+1's load/LN1/QKV chain sits ahead of sample b's
    DVE-heavy tail in every engine's in-order stream;
  - elementwise LN applies / residuals are fused across the 4 channel
    groups (one [128, 2312] op instead of 4) and run on DVE (2-4x modes);
  - LN mean/var -> scale uses rsqrt(v) = exp(-0.5*ln(v*N/(N-1))) on the
    ACT engine (ln+exp share a table set), an 8-op chain instead of a
    ~30-op Newton chain;
  - q/k PSUM evacuations run on ACT (Identity/Copy with scale+bias),
    x/out DMAs on the SP queue, keeping DVE for the big fused ops;
  - qks8 head-split tile packs odd/even heads in partition blocks 0:32 /
    32:64 and is double-buffered so the reshuffle overlaps attention.

Matmul precision: fp8e4m3 with DoubleRow perf mode (2 k-tiles per pass,
2x PE throughput) for QKV / PV / wout / FFN; fp8 DR over two 32-partition
k-tiles for the K=64 attention score matmuls; fp32r/bf16 for the PE
transposes. Weights are pre-scaled by 32 (activations: v by 8, attn-out
by 8 via the softmax-denominator reciprocal) to keep fp8 operands in the
normal range; the inverse scales fold into PSUM-evacuation affine ops.

Softmax denominators come from a fused ones-column in the PV matmul; the
key bias is dropped (softmax shift invariance) and the value bias folded
into an adjusted output-projection bias (bo_eff = bo + bv @ wo).

NOTE: the DMA XBAR transpose (InstDmaTransposeAnt) and Pool SWDGE DMAs
produce wrong results / NaNs on this runtime (CoreSim models both fine),
so NO_XBAR / NO_SWDGE default to the safe PE-transpose / sync-queue
paths.
"""

import os
import sys
import numpy as np

if "/opt/trn_rl_repo" not in sys.path:
    sys.path.insert(0, "/opt/trn_rl_repo")

import concourse.bass as bass
import concourse.tile as tile
from concourse import mybir
from concourse import bass_utils
from concourse.masks import make_identity

F32 = mybir.dt.float32
F32R = mybir.dt.float32r
BF16 = mybir.dt.bfloat16
F8 = mybir.dt.float8e4
AF = mybir.ActivationFunctionType
OP = mybir.AluOpType
DRMODE = mybir.MatmulPerfMode.DoubleRow

# ----------------------------------------------------------------------------
# Workaround for walrus "Too many sync wait commands" on the Tile end-of-kernel
# Drain: split its sem waits across sync-engine NOPs (1 wait each).
# ----------------------------------------------------------------------------
_ORIG_DRAIN = tile.TileContext._drain_and_barrier


def _patched_drain_and_barrier(self, tick_clock, wait_clock):
    from concourse.tile import ScopedClock

    nc = self.nc
    drain_inst = nc.sync.drain()
    wait_clock.add_sem_waits(
        drain_inst.ins, ScopedClock({None: tick_clock.global_clock})
    )
    si = drain_inst.ins.sync_info
    waits = list(si.on_wait or []) if si is not None else []
    if len(waits) > 1:
        drain_inst.ins.sync_info = mybir.SyncInfo(
            on_wait=[], on_update=list(si.on_update or [])
        )
        for i in range(len(waits)):
            nop = nc.sync.nop()
            nop.ins.sync_info = mybir.SyncInfo(on_wait=[waits[i]], on_update=[])
        nc.sync.drain()
    nc.all_engine_barrier()
    popped = nc._tile_sem_poison_stack.pop()
    assert popped is self._sem_poison
    nc.clear_and_free_semaphores(list(self.sems.allocated().values()))
    nc.all_engine_barrier()


tile.TileContext._drain_and_barrier = _patched_drain_and_barrier

# Split excess per-instruction sem waits onto same-engine NOPs: this walrus
# build rejects instructions carrying more than _MAXW sync waits.
_MAXW = int(os.environ.get("BASS_MAXW", "1"))
_orig_add_instruction = tile.TileContext._add_instruction


def _split_add_instruction(self, inst):
    si = getattr(inst, "sync_info", None)
    eng = getattr(inst, "engine", None)
    if (
        si is not None
        and si.on_wait
        and len(si.on_wait) > _MAXW
        and eng is not None
        and eng != mybir.EngineType.Unassigned
    ):
        waits = list(si.on_wait)
        head, tail = waits[:-_MAXW], waits[-_MAXW:]
        for i in range(0, len(head), _MAXW):
            nop = mybir.InstNoOp(
                name=self.nc.get_next_instruction_name(),
                engine=eng,
                sync_info=mybir.SyncInfo(on_wait=head[i : i + _MAXW], on_update=[]),
                bass_nofuse=True,
            )
            _orig_add_instruction(self, nop)
        inst.sync_info = mybir.SyncInfo(
            on_wait=tail, on_update=list(si.on_update or [])
        )
    _orig_add_instruction(self, inst)


tile.TileContext._add_instruction = _split_add_instruction


# Allow using the SBUF beyond the stale 192KB/partition cap (208KB usable).
try:
    import concourse.tile_utils as tile_utils

    tile_utils.max_sbuf_usage = 204 * 1024
except Exception:
    pass

# ----------------------------------------------------------------------------
# Problem constants (hardcoded per the harness contract)
# ----------------------------------------------------------------------------
B, L, D, H, DK, FF = 32, 577, 512, 8, 64, 2048
P = 128
NCORES = 8
NB = B // NCORES          # samples per core
NLT = 5                   # L tiles of 128 (last = 65)
NDT = D // P              # 4
NFT = FF // P             # 16
LTS = [128, 128, 128, 128, 65]
FDP = 578                 # padded free dim for layout-B tiles (even)
NLN = L * D               # layernorm element count
EPS = 1e-6
QKS = float(1.0 / np.sqrt(np.float32(D)))
CH = [(0, 512), (512, 66)]  # free-dim chunks (psum bank = 512 f32)
WS = 32.0                 # fp8 weight pre-scale
VS = 8.0                  # fp8 v pre-scale
OS = 8.0                  # fp8 attn-out pre-scale (folded into 1/denom)

# Feature gates. HW bring-up found the DMA XBAR transpose and Pool SWDGE
# DMAs give wrong results / NaNs on this runtime (CoreSim models both fine),
# so both default to their safe fallbacks: PE transposes + sync-queue DMAs.
NO_XBAR = bool(int(os.environ.get("NO_XBAR", "1")))
NO_LN_ACT = bool(int(os.environ.get("NO_LN_ACT", "0")))
NO_SWDGE = bool(int(os.environ.get("NO_SWDGE", "1")))


def build_nc(nb=NB, reps=1):
    nc = bass.Bass(dynamic_dma_scratch_size=256)
    x_d = nc.dram_tensor("x", (nb, L, D), F32R, kind="ExternalInput")
    out_d = nc.dram_tensor("out", (nb, L, D), F32, kind="ExternalOutput")
    wq_d = nc.dram_tensor("wq", (D, D), F32, kind="ExternalInput")
    wk_d = nc.dram_tensor("wk", (D, D), F32, kind="ExternalInput")
    wv_d = nc.dram_tensor("wv", (D, D), F32, kind="ExternalInput")
    wo_d = nc.dram_tensor("wo", (D, D), F32, kind="ExternalInput")
    bq_d = nc.dram_tensor("bq", (D,), F32, kind="ExternalInput")
    bk_d = nc.dram_tensor("bk", (D,), F32, kind="ExternalInput")  # unused
    bv_d = nc.dram_tensor("bv", (D,), F32, kind="ExternalInput")
    bo_d = nc.dram_tensor("bo", (D,), F32, kind="ExternalInput")
    w1_d = nc.dram_tensor("w1", (D, FF), F32, kind="ExternalInput")
    b1_d = nc.dram_tensor("b1", (FF,), F32, kind="ExternalInput")
    w2_d = nc.dram_tensor("w2", (FF, D), F32, kind="ExternalInput")
    b2_d = nc.dram_tensor("b2", (D,), F32, kind="ExternalInput")
    g1_d = nc.dram_tensor("gamma1", (L * D,), F32R, kind="ExternalInput")
    be1_d = nc.dram_tensor("beta1", (L * D,), F32R, kind="ExternalInput")
    g2_d = nc.dram_tensor("gamma2", (L * D,), F32R, kind="ExternalInput")
    be2_d = nc.dram_tensor("beta2", (L * D,), F32R, kind="ExternalInput")
    _ = bk_d

    with tile.TileContext(nc) as tc:
        from contextlib import ExitStack

        ctx = ExitStack()
        with ctx:
            psA = ctx.enter_context(tc.tile_pool(name="psA", bufs=1, space="PSUM"))
            psO = ctx.enter_context(tc.tile_pool(name="psO", bufs=2, space="PSUM"))
            R = ctx.enter_context(tc.tile_pool(name="res", bufs=1))

            # ---------------- resident tensors ----------------
            ident = R.tile([P, P], F32R)
            identB = R.tile([P, P], BF16)
            ones = R.tile([P, P], F32R)
            sel2 = R.tile([33, P], BF16)
            srAB = R.tile([33, FDP], BF16)

            wq8 = R.tile([P, NDT, D], F8)
            wk8 = R.tile([P, NDT, D], F8)
            wv8 = R.tile([P, NDT, D], F8)
            wo8 = R.tile([P, NDT, D], F8)
            w18 = R.tile([P, NDT, FF], F8)
            w28 = R.tile([P, NFT, D], F8)

            bq_sb = R.tile([P, NDT], F32)
            bv8 = R.tile([P, NDT, 1], F8)
            bo_sb = R.tile([P, NDT], F32)
            b2_sb = R.tile([P, NDT], F32)
            b1_sb = R.tile([P, NFT], F32)
            boe = R.tile([P, NDT], F32)
            nc.sync.dma_start(bq_sb[:], bq_d.rearrange("(o p) -> p o", p=P))
            nc.sync.dma_start(bo_sb[:], bo_d.rearrange("(o p) -> p o", p=P))
            nc.sync.dma_start(b2_sb[:], b2_d.rearrange("(o p) -> p o", p=P))
            nc.sync.dma_start(b1_sb[:], b1_d.rearrange("(o p) -> p o", p=P))

            g1T = R.tile([P, NDT, FDP], BF16)
            be1T = R.tile([P, NDT, FDP], BF16)
            g2T = R.tile([P, NDT, FDP], BF16)
            be2T = R.tile([P, NDT, FDP], BF16)

            # ---------------- preamble (temp pool, freed after) ----------------
            with tc.tile_pool(name="wtmp", bufs=1) as WT:
                identf = WT.tile([P, P], F32, tag="identf")
                make_identity(nc, identf)
                nc.vector.tensor_copy(ident[:], identf[:])
                nc.vector.tensor_copy(identB[:], identf[:])
                onesf = WT.tile([P, P], F32, tag="onesf")
                nc.vector.memset(onesf, 1.0)
                nc.vector.tensor_copy(ones[:], onesf[:])
                sel2f = WT.tile([33, P], F32, tag="sel2f")
                nc.vector.memset(sel2f, 0.0)
                nc.vector.memset(sel2f[0:1, 0:64], 1.0)
                nc.vector.memset(sel2f[32:33, 64:128], 1.0)
                nc.vector.tensor_copy(sel2[:], sel2f[:])
                # rows 1..31 of srAB never written later; keep them finite
                nc.gpsimd.memset(srAB[:], 1.0)

                # attention projection weights -> fp8 * WS
                for w8, w_d in [(wq8, wq_d), (wk8, wk_d), (wv8, wv_d), (wo8, wo_d)]:
                    wr = w_d.rearrange("(ko ki) n -> ki ko n", ki=P)
                    t0 = WT.tile([P, NDT, D], F32, tag="wtmp8")
                    nc.sync.dma_start(t0[:], wr)
                    nc.vector.tensor_scalar_mul(w8[:], t0[:], WS)
                bvt = WT.tile([P, NDT], F32, tag="bvt")
                nc.sync.dma_start(bvt[:], bv_d.rearrange("(o p) -> p o", p=P))
                nc.vector.tensor_scalar_mul(bv8[:, :, 0], bvt[:], WS)

                w1r = w1_d.rearrange("(ko ki) n -> ki ko n", ki=P)
                for kt in range(NDT):
                    t1 = WT.tile([P, FF], F32, tag="wtmp8")
                    nc.sync.dma_start(t1[:], w1r[:, kt, :])
                    nc.vector.tensor_scalar_mul(w18[:, kt, :], t1[:], WS)
                w2r = w2_d.rearrange("(ko ki) n -> ki ko n", ki=P)
                for ft2 in range(0, NFT, 4):
                    t2 = WT.tile([P, 4, D], F32, tag="wtmp8")
                    nc.sync.dma_start(t2[:], w2r[:, ft2 : ft2 + 4, :])
                    nc.vector.tensor_scalar_mul(w28[:, ft2 : ft2 + 4, :], t2[:], WS)

                # gamma/beta -> layout B (PE transposes), cast to bf16.
                # pad col 577: gamma=1, beta=0 so the pad column stays finite.
                nc.gpsimd.memset(g1T[:, :, 577:578], 1.0)
                nc.gpsimd.memset(g2T[:, :, 577:578], 1.0)
                nc.gpsimd.memset(be1T[:, :, 577:578], 0.0)
                nc.gpsimd.memset(be2T[:, :, 577:578], 0.0)
                for src_d, dst in [(g1_d, g1T), (be1_d, be1T), (g2_d, g2T), (be2_d, be2T)]:
                    src2 = src_d.rearrange("(l d) -> l d", d=D)
                    for lt in range(NLT):
                        lsz = LTS[lt]
                        l0 = lt * 128
                        tt = WT.tile([P, D], F32R, tag="ltile")
                        psz = lsz if lsz % 32 == 0 else 96
                        if psz != lsz:
                            nc.vector.memset(tt[64:psz, :].bitcast(mybir.dt.uint32), 0)
                        nc.sync.dma_start(tt[0:lsz, :], src2[l0 : l0 + lsz, :])
                        for dt in range(NDT):
                            pt = psO.tile([P, 1024], F32R, tag="o")
                            nc.tensor.transpose(
                                pt[0:P, 0:psz],
                                tt[0:psz, dt * 128 : (dt + 1) * 128],
                                ident[0:psz, 0:psz],
                            )
                            nc.vector.tensor_copy(
                                dst[:, dt, l0 : l0 + lsz], pt[0:P, 0:lsz].bitcast(F32)
                            )

                # bo_eff = bo + (bv @ wo):  (WS*bv) @ (WS*wo) / WS^2
                for mt in range(NDT):
                    pb = psO.tile([P, 1024], F32, tag="o")
                    for kt in range(NDT):
                        nc.tensor.matmul(
                            pb[:, 0:1],
                            wo8[:, kt, mt * 128 : (mt + 1) * 128],
                            bv8[:, kt, 0:1],
                            start=(kt == 0),
                            stop=(kt == NDT - 1),
                        )
                    nc.vector.tensor_scalar(
                        boe[:, mt : mt + 1], pb[:, 0:1],
                        float(1.0 / (WS * WS)), bo_sb[:, mt : mt + 1],
                        OP.mult, OP.add,
                    )

            p1 = ctx.enter_context(tc.tile_pool(name="p1", bufs=1))
            p1x = ctx.enter_context(tc.tile_pool(name="p1x", bufs=2))
            p1q = ctx.enter_context(tc.tile_pool(name="p1q", bufs=2))
            p2 = ctx.enter_context(tc.tile_pool(name="p2", bufs=2))
            p2b = ctx.enter_context(tc.tile_pool(name="p2b", bufs=2))
            pPR = ctx.enter_context(tc.tile_pool(name="pers", bufs=1))

            def ln_bn(st, t):
                for dt in range(NDT):
                    nc.vector.bn_stats(st[:, dt, 0, :], t[:, dt, 0:512])
                    nc.vector.bn_stats(st[:, dt, 1, :], t[:, dt, 512:577])

            def ln_finish(st):
                # Short-chain LN scale: bn_aggr -> E2 fold -> partition-sum
                # matmul -> rsqrt via exp(-0.5*ln(var*unbias)) on ACT (ln and
                # exp share an activation table set, so no table switch).
                mv = p2.tile([P, 2], F32, tag="mv")
                nc.vector.bn_aggr(mv[:], st[:])
                r2 = p2.tile([P, 2], F32R, tag="r2")
                # r2 = [mean_p, E2_p]:  E2_p = var_p + mean_p^2
                nc.vector.tensor_scalar(
                    r2[:, 1:2], mv[:, 0:1], mv[:, 0:1], mv[:, 1:2], OP.mult, OP.add
                )
                nc.vector.tensor_copy(r2[:, 0:1], mv[:, 0:1])
                ps = psO.tile([P, 1024], F32, tag="o")
                nc.tensor.matmul(ps[:, 0:2], ones, r2[:, 0:2], start=True, stop=True)
                msc = p2.tile([P, 2], F32, tag="msc")
                neg = p2.tile([P, 2], F32, tag="negt")
                # neg = [-mean, -E2] (totals)
                nc.vector.tensor_scalar_mul(neg[:, 0:2], ps[:, 0:2], -1.0 / 128.0)
                # msc[1] = mean^2 - E2 = -var (biased)
                nc.vector.tensor_scalar(
                    msc[:, 1:2], neg[:, 0:1], neg[:, 0:1], neg[:, 1:2], OP.mult, OP.add
                )
                if NO_LN_ACT:
                    # Newton-rsqrt fallback (no Ln table dependency)
                    nc.vector.tensor_scalar_mul(
                        msc[:, 1:2], msc[:, 1:2], -float(NLN / (NLN - 1.0))
                    )
                    v = msc[:, 1:2]
                    y = p2.tile([P, 2], F32, tag="nrt_y")
                    t = p2.tile([P, 2], F32, tag="nrt_t")
                    nc.vector.memset(y[:, 0:1], 1.0)
                    for _it in range(4):
                        nc.vector.tensor_tensor(t[:, 0:1], y[:, 0:1], y[:, 0:1], OP.mult)
                        nc.vector.tensor_tensor(t[:, 0:1], t[:, 0:1], v, OP.mult)
                        nc.vector.tensor_scalar(t[:, 0:1], t[:, 0:1], -0.5, 1.5, OP.mult, OP.add)
                        nc.vector.tensor_tensor(y[:, 0:1], y[:, 0:1], t[:, 0:1], OP.mult)
                    nc.vector.tensor_copy(msc[:, 1:2], y[:, 0:1])
                else:
                    # s = exp(-0.5 * ln(var * N/(N-1))) = 1/sqrt(var_unbiased)
                    lnv = p2.tile([P, 1], F32, tag="lnv")
                    nc.scalar.activation(
                        lnv[:, 0:1], msc[:, 1:2], AF.Ln,
                        scale=-float(NLN / (NLN - 1.0)),
                    )
                    nc.scalar.activation(msc[:, 1:2], lnv[:, 0:1], AF.Exp, scale=-0.5)
                nm = p2.tile([P, 1], F32, tag="negms")
                nc.vector.tensor_tensor(nm[:, 0:1], neg[:, 0:1], msc[:, 1:2], OP.mult)
                return msc, nm

            def new_st():
                return p2.tile([P, NDT, 2, 6], F32, tag="st6", name="st6")

            for _rep in range(reps):
              # persistent across the two phases of one rep
              h2A = pPR.tile([P, nb, NDT, FDP], BF16, tag="h2A")
              g8A = pPR.tile([P, nb, NDT, FDP], F8, tag="g8A")

              # =================== phase ATT ===================
              # Software-pipelined emission: pairs(b) -> front(b+1) -> tail(b)
              # so sample b+1's load/LN1/QKV chain is already in each engine's
              # in-order stream before sample b's DVE-heavy tail, letting the
              # next sample's attention start while the tail drains.
              def emit_front(b):
                # ---- A: load x (layout A); double-buffered so sample b+1's
                # load/cast/transpose/stats overlap sample b's attention ----
                xa = p1x.tile([P, NLT, D], F32R, tag="xa")
                nc.gpsimd.memset(xa[64:128, NLT - 1, :].bitcast(mybir.dt.uint32), 0)
                nc.sync.dma_start(
                    xa[:, 0:4, :],
                    x_d[b, 0:512, :].rearrange("(t p) d -> p t d", p=P),
                )
                nc.sync.dma_start(xa[0:65, 4, :], x_d[b, 512:577, :])

                # ---- B: cast to bf16 (Pool) and transpose via DMA XBAR ----
                # x -> layout B without touching PSUM or PE, so sample b+1's
                # front chain runs entirely off the attention engines.
                xT = p1x.tile([P, NDT, 608], BF16, tag="xT")
                st1 = new_st()
                if NO_XBAR:
                    # cast x to bf16 on the idle Pool engine first: bf16 PE
                    # transposes run 1.5x faster than f32r and the bf16 PSUM
                    # evacuation copies hit the DVE 2x 16-bit mode.
                    xb = p1x.tile([P, NLT, D], BF16, tag="xb")
                    nc.gpsimd.tensor_copy(xb[:, :, :], xa[:, :, :])
                    nc.gpsimd.memset(xT[:, :, 577:608], 0.0)
                    for lt in range(NLT):
                        lsz = LTS[lt]
                        l0 = lt * 128
                        psz = lsz if lsz % 32 == 0 else 96
                        pt = psA.tile([P, 16, 128], BF16, tag="sc")
                        for dt in range(NDT):
                            nc.tensor.transpose(
                                pt[0:P, dt, 0:psz],
                                xb[0:psz, lt, dt * 128 : (dt + 1) * 128],
                                identB[0:psz, 0:psz],
                            )
                        nc.vector.tensor_copy(
                            xT[:, 0:NDT, l0 : l0 + lsz],
                            pt[0:P, 0:NDT, 0:lsz],
                        )
                else:
                    xb = p1x.tile([P, NLT, D], BF16, tag="xb")
                    nc.gpsimd.tensor_copy(xb[:, :, :], xa[:, :, :])
                    for lt in range(4):
                        nc.sync.dma_start(
                            xT[:, 0:NDT, lt * 128 : lt * 128 + 128],
                            xb[:, lt, :],
                            transpose=True,
                        )
                    nc.sync.dma_start(
                        xT[:, 0:NDT, 512:608], xb[0:96, 4, :], transpose=True
                    )
                ln_bn(st1, xT)

                # ---- C: LN1 -> hq (fp8), fused over channel groups ----
                msc1, nm1 = ln_finish(st1)
                hb = p1.tile([P, NDT, FDP], BF16, tag="hb")
                hq = p1.tile([P, NDT, 640], F8, tag="hq")  # 640: 64B-aligned pair stride for dual-fp8 ldweights
                nc.vector.tensor_scalar(
                    hb[:, :, :], xT[:, :, 0:FDP],
                    msc1[:, 1:2], nm1[:, 0:1], OP.mult, OP.add,
                )
                nc.vector.tensor_tensor(hb[:, :, :], hb[:, :, :], g1T[:, :, :], OP.mult)
                nc.vector.tensor_tensor(
                    hq[:, 0:NDT, 0:FDP], hb[:, :, :], be1T[:, :, :], OP.add
                )

                # ---- D: QKV (fp8 DoubleRow) ----
                # q/k are evacuated to fp8, then partition-split by DMA into
                # a [32, 2(j), .] layout so the K=64 score matmuls can run as
                # fp8 DoubleRow over two 32-partition k-tiles (2x PE rate).
                qkT = p1.tile([P, 2, NDT, FDP], F8, tag="qkT")
                # qks8[32*(h%2) + sub, ip, h//2, j, l]: odd/even heads in
                # partition blocks 0:32 / 32:64 (q,k share a base partition as
                # the PE requires); double-buffered so sample b+1's reshuffle
                # overlaps sample b's attention instead of WAR-stalling on it.
                qks8 = p1q.tile([64, 2, NDT, 2, 640], F8, tag="qks8")
                for ip, w8 in enumerate([wq8, wk8]):
                    for mt in range(NDT):
                        ps = psO.tile([P, 1024], F32, tag="o")
                        for kp in range(2):
                            for c0, csz in CH:
                                nc.tensor.matmul(
                                    ps[:, c0 : c0 + csz],
                                    w8[:, 2 * kp : 2 * kp + 2, mt * 128 : (mt + 1) * 128],
                                    hq[:, 2 * kp : 2 * kp + 2, c0 : c0 + csz],
                                    start=(kp == 0),
                                    stop=(kp == 1),
                                    perf_mode=DRMODE,
                                )
                        # evacuate on DVE: the ACT engine is the attention
                        # bottleneck (exp), keep it clear of affine evacs
                        if ip == 0:
                            nc.vector.tensor_scalar(
                                qkT[:, 0, mt, 0:FDP], ps[:, 0:FDP],
                                float(1.0 / WS), bq_sb[:, mt : mt + 1],
                                OP.mult, OP.add,
                            )
                        else:
                            nc.vector.tensor_scalar_mul(
                                qkT[:, 1, mt, 0:FDP], ps[:, 0:FDP], float(1.0 / WS)
                            )
                # head-split reshuffle on the Pool SWDGE queue (25ns seq hold;
                # keeps the SP/Act queues free for loads and activations)
                dma_q = nc.sync if NO_SWDGE else nc.gpsimd
                for ip in range(2):
                    for h01 in range(2):
                        for j in range(2):
                            p0 = 64 * h01 + 32 * j
                            d0 = 32 * h01
                            dma_q.dma_start(
                                qks8[d0 : d0 + 32, ip, 0:NDT, j, 0:FDP],
                                qkT[p0 : p0 + 32, ip, 0:NDT, 0:FDP],
                            )

                v8 = p1.tile([P, NLT, H, 128], F8, tag="v8")  # 128: aligned dual-fp8 ldweights stride/offset
                nc.gpsimd.memset(v8[:, :, :, 64:66], 1.0)
                for mt in range(NLT):
                    lsz = LTS[mt]
                    lpz = lsz if lsz % 2 == 0 else lsz + 1
                    l0 = mt * 128
                    ps = psO.tile([P, 1024], F32, tag="o")
                    for kp in range(2):
                        nc.tensor.matmul(
                            ps[0:lpz, 0:512],
                            hq[:, 2 * kp : 2 * kp + 2, l0 : l0 + lpz],
                            wv8[:, 2 * kp : 2 * kp + 2, :],
                            start=(kp == 0),
                            stop=(kp == 1),
                            perf_mode=DRMODE,
                        )
                    nc.scalar.activation(
                        v8[0:lsz, mt, :, 0:64], ps[0:lsz, 0:512], AF.Copy,
                        scale=float(VS / WS),
                    )
                return {"xT": xT, "qks8": qks8, "v8": v8}

              def emit_pairs(b, S):
                qks8 = S["qks8"]
                v8 = S["v8"]

                # ---- E: attention, software-pipelined over head pairs ----
                # Scores for both heads of a pair land in one [P, 2, 1024]
                # PSUM tile so exp is a single fused ACT instruction per L
                # tile. PV matmuls for the PREVIOUS pair are interleaved into
                # the score loop to keep the PE busy while ACT runs exp
                # (the "sc" psum tile is single-buffered). Softmax
                # normalization: DVE reciprocal straight off the PV psum
                # denominator row, partition-broadcast to 128 rows via an
                # SBUF->SBUF DMA (stride-0 source), then one DVE multiply
                # per head.
                oT = p1.tile([P, NDT, FDP], F8, tag="oT")

                def emit_pv_group(hp, expT, psos, h01, c0, csz):
                    h = 2 * hp + h01
                    pso = psos[h01]
                    for kp in range(2):
                        nc.tensor.matmul(
                            pso[0:66, c0 : c0 + csz],
                            v8[:, 2 * kp : 2 * kp + 2, h, 0:66],
                            expT[:, h01, 2 * kp : 2 * kp + 2, c0 : c0 + csz],
                            start=(kp == 0),
                            stop=False,
                            perf_mode=DRMODE,
                        )
                    nc.tensor.matmul(
                        pso[0:66, c0 : c0 + csz],
                        v8[0:65, 4, h, 0:66],
                        expT[0:65, h01, 4, c0 : c0 + csz],
                        start=False,
                        stop=True,
                    )

                def att_qk_exp(hp, prev_expT, prev_psos):
                    # scores+exp for pair hp; PV for pair hp-1 interleaved
                    expT = p2b.tile([P, 2, NLT, FDP], F8, tag="expT")
                    pv_slots = (
                        [(0, 0, 512), (0, 512, 66), (1, 0, 512), (1, 512, 66)]
                        if prev_expT is not None else []
                    )
                    for mt in range(NLT):
                        lsz = LTS[mt]
                        lpz = lsz if lsz % 2 == 0 else lsz + 1
                        l0 = mt * 128
                        sc = psA.tile([P, 2, 1024], F32, tag="sc")
                        for h01 in range(2):
                            h = 2 * hp + h01
                            bp = 32 * h01
                            for c0, csz in CH:
                                nc.tensor.matmul(
                                    sc[0:lpz, h01, c0 : c0 + csz],
                                    qks8[bp : bp + 32, 1, h // 2, 0:2, l0 : l0 + lpz],
                                    qks8[bp : bp + 32, 0, h // 2, 0:2, c0 : c0 + csz],
                                    start=True,
                                    stop=True,
                                    perf_mode=DRMODE,
                                )
                        if mt > 0 and pv_slots:
                            h01, c0, csz = pv_slots.pop(0)
                            emit_pv_group(hp - 1, prev_expT, prev_psos, h01, c0, csz)
                        nc.scalar.activation(
                            expT[0:lsz, 0:2, mt, 0:FDP],
                            sc[0:lsz, 0:2, 0:FDP],
                            AF.Exp, scale=QKS,
                        )
                        last_sc = sc
                    while pv_slots:
                        h01, c0, csz = pv_slots.pop(0)
                        emit_pv_group(hp - 1, prev_expT, prev_psos, h01, c0, csz)
                    return expT, last_sc

                def new_psos():
                    return [psO.tile([P, 1024], F32, tag="o", name="pso")
                            for _ in range(2)]

                def att_norm(hp, psos, scratch_sc):
                    with nc.allow_low_precision(reason="softmax denom recip"):
                        nc.vector.reciprocal(srAB[0:1, 0:FDP], psos[0][64:65, 0:FDP])
                        nc.vector.reciprocal(srAB[32:33, 0:FDP], psos[1][64:65, 0:FDP])
                    prb = scratch_sc[:, 0, :]
                    for c0, csz in CH:
                        nc.tensor.matmul(
                            prb[:, c0 : c0 + csz],
                            sel2[0:33, 0:128],
                            srAB[0:33, c0 : c0 + csz],
                            start=True,
                            stop=True,
                        )
                    rb = p2.tile([P, FDP], F32, tag="rb")
                    nc.vector.tensor_copy(rb[:, 0:FDP], prb[:, 0:FDP])
                    for h01 in range(2):
                        pb = 64 * h01
                        nc.vector.tensor_tensor(
                            oT[pb : pb + 64, hp, 0:FDP],
                            psos[h01][0:64, 0:FDP],
                            rb[pb : pb + 64, 0:FDP],
                            OP.mult,
                        )

                prev_expT = None
                prev_psos = None
                for hp in range(H // 2):
                    expT, last_sc = att_qk_exp(hp, prev_expT, prev_psos)
                    if prev_psos is not None:
                        att_norm(hp - 1, prev_psos, last_sc)
                    psos = new_psos()
                    prev_expT, prev_psos = expT, psos
                # last pair's PV + norm
                for h01 in range(2):
                    for c0, csz in CH:
                        emit_pv_group(H // 2 - 1, prev_expT, prev_psos, h01, c0, csz)
                sc_fin = psA.tile([P, 2, 1024], F32, tag="sc")
                att_norm(H // 2 - 1, prev_psos, sc_fin)
                return oT

              def emit_tail(b, S, oT):
                h2T = h2A[:, b]
                g8 = g8A[:, b]
                xT = S["xT"]

                # ---- F: output projection (fp8 DR) + residual, LN2 stats ----
                st2 = new_st()
                for mt in range(NDT):
                    ps = psO.tile([P, 1024], F32, tag="o")
                    for kp in range(2):
                        for c0, csz in CH:
                            nc.tensor.matmul(
                                ps[:, c0 : c0 + csz],
                                wo8[:, 2 * kp : 2 * kp + 2, mt * 128 : (mt + 1) * 128],
                                oT[:, 2 * kp : 2 * kp + 2, c0 : c0 + csz],
                                start=(kp == 0),
                                stop=(kp == 1),
                                perf_mode=DRMODE,
                            )
                    nc.vector.tensor_scalar(
                        h2T[:, mt, 0:FDP], ps[:, 0:FDP],
                        float(1.0 / (WS * OS)), boe[:, mt : mt + 1],
                        OP.mult, OP.add,
                    )
                # fused residual add over all channel groups, then LN2 stats
                nc.vector.tensor_tensor(
                    h2T[:, :, :], h2T[:, :, :], xT[:, :, 0:FDP], OP.add
                )
                ln_bn(st2, h2T)

                # ---- G: LN2 -> g8 (fp8), fused over channel groups ----
                msc2, nm2 = ln_finish(st2)
                gb = p1.tile([P, NDT, FDP], BF16, tag="gb")
                nc.vector.tensor_scalar(
                    gb[:, :, :], h2T[:, :, :],
                    msc2[:, 1:2], nm2[:, 0:1], OP.mult, OP.add,
                )
                nc.vector.tensor_tensor(gb[:, :, :], gb[:, :, :], g2T[:, :, :], OP.mult)
                nc.vector.tensor_tensor(g8[:, :, :], gb[:, :, :], be2T[:, :, :], OP.add)

              Sx = emit_front(0)
              for b in range(nb):
                  oT = emit_pairs(b, Sx)
                  Sn = emit_front(b + 1) if b + 1 < nb else None
                  emit_tail(b, Sx, oT)
                  Sx = Sn

              # =================== phase FFN ===================
              for b in range(nb):
                h2T = h2A[:, b]
                g8 = g8A[:, b]

                # ---- H: FFN (fp8 DR) ----
                ff8 = p1.tile([P, NFT, FDP], F8, tag="ff8")
                for ft in range(NFT):
                    ps = psO.tile([P, 1024], F32, tag="o")
                    for kp in range(2):
                        for c0, csz in CH:
                            nc.tensor.matmul(
                                ps[:, c0 : c0 + csz],
                                w18[:, 2 * kp : 2 * kp + 2, ft * 128 : (ft + 1) * 128],
                                g8[:, 2 * kp : 2 * kp + 2, c0 : c0 + csz],
                                start=(kp == 0),
                                stop=(kp == 1),
                                perf_mode=DRMODE,
                            )
                    nc.scalar.activation(
                        ff8[:, ft, 0:FDP], ps[:, 0:FDP],
                        # SIM_GELU: CoreSim lacks Gelu; debug path emits Tanh
                        # and the sim harness patches np.tanh to exact gelu.
                        AF.Tanh if os.environ.get("SIM_GELU") else AF.Gelu,
                        bias=b1_sb[:, ft : ft + 1], scale=float(1.0 / WS),
                    )
                outT = p1.tile([P, NDT, 608], BF16, tag="outT")
                nc.gpsimd.memset(outT[:, :, 578:608].bitcast(mybir.dt.uint32), 0)
                for mt in range(NDT):
                    ps = psO.tile([P, 1024], F32, tag="o")
                    for fp in range(NFT // 2):
                        for c0, csz in CH:
                            nc.tensor.matmul(
                                ps[:, c0 : c0 + csz],
                                w28[:, 2 * fp : 2 * fp + 2, mt * 128 : (mt + 1) * 128],
                                ff8[:, 2 * fp : 2 * fp + 2, c0 : c0 + csz],
                                start=(fp == 0),
                                stop=(fp == NFT // 2 - 1),
                                perf_mode=DRMODE,
                            )
                    nc.vector.tensor_scalar(
                        outT[:, mt, 0:FDP], ps[:, 0:FDP],
                        float(1.0 / WS), b2_sb[:, mt : mt + 1],
                        OP.mult, OP.add,
                    )
                nc.vector.tensor_tensor(
                    outT[:, 0:NDT, 0:FDP], outT[:, 0:NDT, 0:FDP], h2T[:, :, :], OP.add
                )

                # ---- I: transpose back to layout A + store ----
                # lt 0..3 batched into one PSUM tile -> single evacuation copy
                oA = p1.tile([P, NLT, D], F32, tag="oA")
                pt = psA.tile([P, 16, 128], BF16, tag="sc")
                for lt in range(4):
                    for dt in range(NDT):
                        nc.tensor.transpose(
                            pt[0:128, lt * NDT + dt, 0:128],
                            outT[:, dt, lt * 128 : lt * 128 + 128],
                            identB,
                        )
                nc.vector.tensor_copy(oA[0:128, 0:4, :], pt[0:128, 0:16, 0:128])
                pt2 = psA.tile([P, 16, 128], BF16, tag="sc")
                for dt in range(NDT):
                    nc.tensor.transpose(
                        pt2[0:96, dt, 0:128], outT[:, dt, 512 : 512 + 96], identB
                    )
                nc.vector.tensor_copy(oA[0:65, 4, :], pt2[0:65, 0:NDT, 0:128])
                nc.sync.dma_start(
                    out_d[b, 0:512, :].rearrange("(t p) d -> p t d", p=P),
                    oA[:, 0:4, :],
                )
                nc.sync.dma_start(out_d[b, 512:577, :], oA[0:65, 4, :])

    return nc


_NC_CACHE = {}
LAST_RESULTS = None


def _get_nc(nb=NB):
    if nb not in _NC_CACHE:
        _NC_CACHE[nb] = build_nc(nb)
    return _NC_CACHE[nb]


def kernel(**inputs):
    x = np.ascontiguousarray(np.asarray(inputs["x"], dtype=np.float32))
    assert x.shape == (B, L, D), x.shape
    weights = {
        k: np.ascontiguousarray(np.asarray(inputs[k], dtype=np.float32))
        for k in [
            "wq", "wk", "wv", "wo", "bq", "bk", "bv", "bo",
            "w1", "b1", "w2", "b2", "gamma1", "beta1", "gamma2", "beta2",
        ]
    }
    nc = _get_nc(NB)
    in_maps = []
    for i in range(NCORES):
        m = {"x": x[i * NB : (i + 1) * NB]}
        m.update(weights)
        in_maps.append(m)
    res = bass_utils.run_bass_kernel_spmd(nc, in_maps, core_ids=list(range(NCORES)))
    global LAST_RESULTS
    LAST_RESULTS = res
    out = np.concatenate([res.results[i]["out"] for i in range(NCORES)], axis=0)
    return out.astype(np.float32)

